# revision 31
# baseline (speedup 1.0000x reference)
"""HOG generator kernel for Trainium2, data-parallel over 8 NeuronCores.

v2 architecture (validated numerically in numpy against the jax reference):
  - Per chunk (2 images x row-half [113, 2x224]): horizontal Sobel parts
    D = x(l)-x(r), S = x(l)+2x(c)+x(r) on GPSIMD from a halo-padded X tile.
  - All orientation comparisons moved to the PE as fp32r matmuls:
    u_k = gx - tan_k*gy is linear, so each bin plane is two accumulated
    matmuls (vertical band matrices, tan_k folded into constant weights).
    sign(q_k) = sign(gx)*sign(u_k); the gx==0 reflect-edge columns are
    handled exactly via a q~ = gx + 1e-20*gy plane (sign(q~)=sign(gy) there).
  - Masks via ACT Sign (in the same table set as Sqrt/Square/Copy ->
    no ACT_TABLE_LOAD thrash; one table set for the whole kernel).
    mask = (1+s)/2 algebra: bin histogram = adjacent diffs of signed pooled
    sums plus the unmasked total; the global factor 2 cancels in the final
    L2 normalization.
  - Column 8:1 pooling: bf16 tree adds on DVE; row pooling: PE matmul with a
    replicated pool matrix so each chunk's [14, 504] result lands at its own
    partition offset; the [112, x] batched final pass (diff/square/norm)
    then runs at full lane utilization.
"""
import math
import sys

import numpy as np

sys.path.insert(0, "/opt/trn_rl_repo")

import concourse.bass as bass
import concourse.bacc as bacc
import concourse.mybir as mybir
from concourse import tile
from concourse.bass_utils import run_bass_kernel_spmd

N_CORES = 8
IMGS_PER_CORE = 16
NB = 9
F32 = mybir.dt.float32
F32R = mybir.dt.float32r
BF16 = mybir.dt.bfloat16
AF = mybir.ActivationFunctionType
OP = mybir.AluOpType
AX = mybir.AxisListType.X
TANS = [math.tan(k * math.pi / 9.0) for k in range(1, 9)]
EPS_SGN = 1e-20

# matmul-weight const blob (float32r) column layout, rows 0:113
_C = {}
_o = 0
for _nm in ("ls0", "ls1", "lsn0", "lsn1", "ld0", "ld1", "eps0", "eps1"):
    _C[_nm] = _o
    _o += 112
_C["w0"] = _o          # 8 tan-matrices for ch 0
_o += 8 * 112
_C["w1"] = _o          # 8 tan-matrices for ch 1
_o += 8 * 112
_C["poolm"] = _o       # 8 slot-selective pool matrices
_o += 8 * 112
CWW = _o
# plain-f32 const blob (gaussian factors)
_C["gc"] = 0
_C["gr2"] = 224
CWF = 225


def _band(ch, vec):
    """Vertical banded conv matrix [113, 112] with reflect padding folded."""
    m = np.zeros((113, 112), np.float64)
    for i in range(112):
        for d in range(3):
            if ch == 0:
                r = i - 1 + d
                if r == -1:
                    r = 1
            else:
                r = i + d
                if r == 113:
                    r = 111
            m[r, i] += vec[d]
    return m


def _host_constants(weight_x, gaussian_kernel):
    wx = np.asarray(weight_x, np.float32).reshape(3, 3)
    v_s = wx[:, 0].astype(np.float64)          # [1,2,1]
    v_d = wx[0, :].astype(np.float64)          # [1,0,-1]
    g2 = np.asarray(gaussian_kernel, np.float64).reshape(16, 16)
    wt = np.sqrt(np.diag(g2))                  # g2[i,j] == wt[i]*wt[j]

    blobw = np.zeros((113, CWW), np.float32)
    for ch in (0, 1):
        ls = _band(ch, v_s)
        ld = _band(ch, v_d)
        blobw[:, _C[f"ls{ch}"]:_C[f"ls{ch}"] + 112] = ls
        blobw[:, _C[f"lsn{ch}"]:_C[f"lsn{ch}"] + 112] = -ls
        blobw[:, _C[f"ld{ch}"]:_C[f"ld{ch}"] + 112] = ld
        blobw[:, _C[f"eps{ch}"]:_C[f"eps{ch}"] + 112] = EPS_SGN * ld
        for k in range(1, 9):
            s = -TANS[k - 1] if k <= 4 else TANS[k - 1]
            o = _C[f"w{ch}"] + (k - 1) * 112
            blobw[:, o:o + 112] = s * ld
    for slot in range(8):
        pm = np.zeros((113, 112), np.float32)
        for q in range(112):
            pm[q, slot * 14 + q // 8] = 1.0
        o = _C["poolm"] + slot * 112
        blobw[:, o:o + 112] = pm
    blobf = np.zeros((113, CWF), np.float32)
    blobf[0:112, _C["gc"]:_C["gc"] + 224] = wt[np.arange(224) % 16][None, :]
    blobf[0:112, _C["gr2"]] = wt[np.arange(112) % 16] ** 2
    return {"consts_w": blobw, "consts_f": blobf}


def _rep(ap, n, pos=1):
    """Insert a broadcast (step-0) dim of size n into an AP at free pos."""
    l = [list(d) for d in ap.ap]
    l.insert(pos, [0, n])
    return bass.AP(ap.tensor, ap.offset, l)


def build_program(n_img=IMGS_PER_CORE):
    assert n_img == 16
    nc = bacc.Bacc("TRN2", debug=False)
    x_d = nc.dram_tensor("x", [n_img, 224, 224], F32, kind="ExternalInput").ap()
    cw_d = nc.dram_tensor("consts_w", [113, CWW], F32R,
                          kind="ExternalInput").ap()
    cf_d = nc.dram_tensor("consts_f", [113, CWF], F32,
                          kind="ExternalInput").ap()
    out_d = nc.dram_tensor("out", [n_img, 28, NB, 28], F32,
                           kind="ExternalOutput").ap()

    with tile.TileContext(nc) as tc:
        with (
            tc.tile_pool(name="const", bufs=1) as cp,
            tc.tile_pool(name="work", bufs=3) as wp,
            tc.tile_pool(name="keep", bufs=1) as kp,
            tc.tile_pool(name="fin", bufs=2) as fp,
            tc.tile_pool(name="psum", bufs=1, space="PSUM") as pp,
        ):
            CTW = cp.tile([113, CWW], F32R, tag="CTW")
            nc.sync.dma_start(CTW[:, :], cw_d)
            CTF = cp.tile([113, CWF], F32, tag="CTF")
            nc.sync.dma_start(CTF[:, :], cf_d)

            def wslice(nm, ch, k=0):
                if nm == "w":
                    o = _C[f"w{ch}"] + (k - 1) * 112
                else:
                    o = _C[f"{nm}{ch}"]
                return CTW[:, o:o + 112]

            def poolm_r(slot):
                o = _C["poolm"] + slot * 112
                return CTW[0:112, o:o + 112]
            gc_ap = CTF[0:112, _C["gc"]:_C["gc"] + 224]
            gr2_ap = CTF[0:112, _C["gr2"]:_C["gr2"] + 1]

            Ps_all = kp.tile([112, 2, NB, 2, 28], F32, tag="Ps_all")

            for q in range(16):            # chunk = (image pair, row half)
                i0 = (q // 2) * 2
                ch = q % 2
                big, slot = q // 8, q % 8
                r0 = 0 if ch == 0 else 111

                X = wp.tile([113, 2, 226], F32, tag="X", bufs=4)
                nc.gpsimd.dma_start(X[:, 0, 1:225], x_d[i0, r0:r0 + 113, :])
                nc.gpsimd.dma_start(X[:, 1, 1:225], x_d[i0 + 1, r0:r0 + 113, :])
                nc.gpsimd.tensor_copy(X[:, :, 0:1], X[:, :, 2:3])
                nc.gpsimd.tensor_copy(X[:, :, 225:226], X[:, :, 223:224])

                D = wp.tile([113, 2, 224], F32R, tag="D", bufs=4)
                nc.gpsimd.tensor_sub(D[:, :, :], X[:, :, 0:224], X[:, :, 2:226])
                Sf = wp.tile([113, 2, 224], F32, tag="Sf", bufs=4)
                nc.gpsimd.tensor_add(Sf[:, :, :], X[:, :, 0:224], X[:, :, 2:226])
                S = wp.tile([113, 2, 224], F32R, tag="S", bufs=4)
                nc.vector.scalar_tensor_tensor(
                    S[:, :, :], X[:, :, 1:225], 2.0, Sf[:, :, :],
                    OP.mult, OP.add)

                D_r = D[:, :, :]
                S_r = S[:, :, :]

                # PSUM: single-plane groups (tag U1, bufs=7 -> 7 banks) give
                # the PE a 7-plane runway ahead of the Sign extractions; plus
                # the cross-chunk pool accumulator PSB (1 bank) = 16K exactly.
                # Plane k: +L_s@D for k<=4 (and the q~ plane 0), -L_s@D for
                # k>=5, then the tan-weight (or eps) S matmul accumulates.
                su = wp.tile([112, NB, 448], BF16, tag="su", bufs=4)

                # gy first so the magnitude chain (C->S2->mg->magG16->mgs)
                # overlaps the bin-plane matmuls instead of trailing them.
                Ggy = pp.tile([112, 512], F32, tag="U1", bufs=7)
                nc.tensor.matmul(Ggy[:, 0:448], wslice("ld", ch), S_r,
                                 start=True, stop=True)
                C = wp.tile([112, 448], F32, tag="C")
                nc.scalar.activation(C[:, :], Ggy[:, 0:448], AF.Square)

                for k in range(9):
                    G = pp.tile([112, 512], F32, tag="U1", bufs=7)
                    nc.tensor.matmul(G[:, 0:448],
                                     wslice("ls" if k <= 4 else "lsn", ch),
                                     D_r, start=True, stop=False)
                    nc.tensor.matmul(G[:, 0:448],
                                     wslice("eps", ch) if k == 0 else
                                     wslice("w", ch, k), S_r,
                                     start=False, stop=True)
                    nc.scalar.activation(su[:, k, :], G[:, 0:448], AF.Sign)
                    if k == 0:
                        A = wp.tile([112, 448], F32, tag="A")
                        nc.scalar.activation(A[:, :], G[:, 0:448], AF.Square)

                S2 = wp.tile([112, 448], F32, tag="S2")
                nc.gpsimd.tensor_add(S2[:, :], A[:, :], C[:, :])
                mg = wp.tile([112, 448], F32, tag="mg")
                nc.scalar.activation(mg[:, :], S2[:, :], AF.Sqrt, scale=gr2_ap)
                magG16 = wp.tile([112, 2, 224], BF16, tag="magG16")
                nc.vector.tensor_mul(magG16[:, :, :],
                                     mg[:, :].rearrange("p (i c) -> p i c", i=2),
                                     _rep(gc_ap, 2))
                mgs = wp.tile([112, 2, 224], BF16, tag="mgs")
                nc.vector.tensor_mul(mgs[:, :, :], magG16[:, :, :],
                                     su[:, 0, :].rearrange("p (i c) -> p i c",
                                                           i=2))

                CP = wp.tile([112, NB, 2, 28], F32R, tag="CP", bufs=4)
                mgs_e = mgs[:, :, :].rearrange("p i (c e) -> p i c e", e=8)
                for half, base, k0 in ((0, 1, 1), (1, 5, 5)):
                    sub = su[:, base:base + 4, :].rearrange(
                        "p k (i c e) -> p k i c e", i=2, e=8)
                    Tt = wp.tile([112, 4, 2, 28, 8], BF16, tag=f"Tt{half}")
                    nc.vector.tensor_mul(Tt[:, :, :, :, :], sub,
                                         _rep(mgs_e, 4, pos=1))
                    t1 = wp.tile([112, 4, 2, 28, 4], BF16, tag=f"t1{half}")
                    nc.vector.tensor_add(t1[:, :, :, :, :],
                                         Tt[:, :, :, :, 0:4],
                                         Tt[:, :, :, :, 4:8])
                    t2 = wp.tile([112, 4, 2, 28, 2], F32, tag=f"t2{half}")
                    nc.vector.tensor_add(t2[:, :, :, :, :],
                                         t1[:, :, :, :, 0:2],
                                         t1[:, :, :, :, 2:4])
                    nc.vector.tensor_add(CP[:, k0:k0 + 4, :, :],
                                         t2[:, :, :, :, 0],
                                         t2[:, :, :, :, 1])
                with nc.allow_low_precision(reason="f32r CP for pool matmul"):
                    nc.vector.reduce_sum(CP[:, 0, :, :],
                                         magG16[:, :, :].rearrange(
                                             "p i (c e) -> p i c e", e=8),
                                         axis=AX)

                # row pool: slot-selective matrix accumulates this chunk's
                # [14, 504] block into its partition slot of the shared bank.
                if slot == 0:
                    PSB = pp.tile([112, 512], F32, tag="PSB")
                nc.tensor.matmul(PSB[:, 0:504], poolm_r(slot), CP[:, :, :, :],
                                 start=(slot == 0), stop=(slot == 7))
                if slot == 7:
                    nc.scalar.activation(
                        Ps_all[:, big, :, :, :],
                        PSB[:, 0:504].rearrange("p (k i c) -> p k i c",
                                                k=NB, i=2),
                        AF.Copy)

            # batched final pass: diffs, L2 norm, output
            for big in range(2):
                Pv = Ps_all[:, big, :, :, :]
                hh = fp.tile([112, NB, 2, 28], F32, tag="hh")
                nc.vector.tensor_sub(hh[:, 0:8, :, :], Pv[:, 0:8, :, :],
                                     Pv[:, 1:9, :, :])
                nc.vector.tensor_add(hh[:, 8, :, :], Pv[:, 8, :, :],
                                     Pv[:, 0, :, :])
                sq = fp.tile([112, NB, 2, 28], F32, tag="sq")
                nc.vector.tensor_mul(sq[:, :, :, :], hh[:, :, :, :],
                                     hh[:, :, :, :])
                ss = fp.tile([112, 2, 28], F32, tag="ss")
                nc.vector.reduce_sum(ss[:, :, :],
                                     sq[:, :, :, :].rearrange(
                                         "p k i c -> p i c k"), axis=AX)
                nrm = fp.tile([112, 2, 28], F32, tag="nrm")
                nc.scalar.activation(nrm[:, :, :], ss[:, :, :], AF.Sqrt)
                nc.vector.tensor_scalar_max(nrm[:, :, :], nrm[:, :, :], 1e-12)
                inv = fp.tile([112, 2, 28], F32, tag="inv")
                nc.vector.reciprocal(inv[:, :, :], nrm[:, :, :])
                ov = fp.tile([112, NB, 2, 28], F32, tag="ov")
                nc.vector.tensor_mul(ov[:, :, :, :], hh[:, :, :, :],
                                     _rep(inv[:, :, :], NB, pos=1))
                for qq in range(4):
                    i0 = big * 8 + 2 * qq
                    nc.gpsimd.dma_start(
                        out_d[i0:i0 + 2, :, :, :].rearrange(
                            "i (h r) k c -> (h r) k i c", r=14),
                        ov[28 * qq:28 * qq + 28, :, :, :])
    nc.compile()
    return nc


def _install_ntff_shim():
    """Provide antenv.axon_hooks (absent in this image) so trace=True works."""
    import sys as _sys
    if "antenv.axon_hooks" in _sys.modules:
        return
    import contextlib
    import ctypes
    import types

    so_path = "/opt/axon/libaxon_pjrt.so"
    lib = ctypes.CDLL(so_path)
    if not hasattr(lib, "axon_start_nrt_profile"):
        hook = None
    else:
        lib.axon_start_nrt_profile.argtypes = [
            ctypes.POINTER(ctypes.c_int64), ctypes.c_size_t]
        lib.axon_start_nrt_profile.restype = ctypes.c_int64
        lib.axon_stop_nrt_profile.argtypes = [ctypes.c_char_p]
        lib.axon_stop_nrt_profile.restype = ctypes.c_int64

        @contextlib.contextmanager
        def hook(output_dir, device_ids):
            import jax
            jax.devices()
            if device_ids:
                ids = (ctypes.c_int64 * len(device_ids))(*device_ids)
                rc = lib.axon_start_nrt_profile(ids, len(device_ids))
            else:
                rc = lib.axon_start_nrt_profile(None, 0)
            if rc != 0:
                raise RuntimeError(f"axon_start_nrt_profile rc={rc}")
            try:
                yield
            finally:
                n = lib.axon_stop_nrt_profile(str(output_dir).encode())
                print(f"profile: {n} file(s) written to {output_dir}",
                      file=sys.stderr)

    mod = types.ModuleType("antenv.axon_hooks")
    mod._hook = hook
    mod.get_axon_ntff_profile_hook = lambda: mod._hook
    mod.set_axon_ntff_profile_hook = lambda h: setattr(mod, "_hook", h)
    _sys.modules["antenv.axon_hooks"] = mod


_prog_cache = {}


def _get_prog(n_img):
    if n_img not in _prog_cache:
        _prog_cache[n_img] = build_program(n_img)
    return _prog_cache[n_img]


def kernel(x, weight_x, weight_y, gaussian_kernel, _trace=False):
    x = np.ascontiguousarray(np.asarray(x, np.float32).reshape(128, 224, 224))
    consts = _host_constants(weight_x, gaussian_kernel)
    nc = _get_prog(IMGS_PER_CORE)
    in_maps = []
    for c in range(N_CORES):
        m = {"x": x[c * IMGS_PER_CORE:(c + 1) * IMGS_PER_CORE]}
        m.update(consts)
        in_maps.append(m)
    if _trace:
        _install_ntff_shim()
    res = run_bass_kernel_spmd(nc, in_maps, core_ids=list(range(N_CORES)),
                               trace=_trace)
    outs = [r["out"] for r in res.results]            # (16, 28, 9, 28) each
    full = np.concatenate(outs, axis=0)               # (128, 28, 9, 28)
    feat = full.transpose(0, 2, 1, 3)                 # (b, 9, 28, 28)
    feat = feat.transpose(0, 2, 3, 1)                 # (b, 28, 28, 9)
    feat = feat.reshape(128, 14, 2, 14, 2, NB)
    feat = feat.transpose(0, 1, 3, 5, 2, 4).reshape(128, 196, NB * 4)
    if _trace:
        return np.ascontiguousarray(feat), res
    return np.ascontiguousarray(feat)
